# Initial kernel scaffold
#
"""Trainium2 Bass kernel for nn_AttentionLayer (sparse_attention, 8-core head-parallel).

Reference computation (B=4, S=16, H=16, D=128, HID=2048, P=8192):
    qkv = x @ w_qkv + b_qkv ; split into q,k,v
    k_full = concat(cached_k broadcast over batch, new k)   # [B,H,P+S,D]
    out = softmax(q @ k_full^T / sqrt(D)) @ v_full
    y = out @ w_proj + b_proj

Sharding: tensor-parallel over heads. Each of the 8 cores owns 2 heads:
column-sharded w_qkv/b_qkv (its heads' q,k,v columns), the head slice of the
KV cache, and the row slice of w_proj. Each core emits a partial y
[64, 2048]; the unshard step sums the 8 partials and adds b_proj (row-parallel
linear with host-side reduction).

Device-side layout choices (all prepared on host, free w.r.t. HW exec time):
  - x is passed pre-transposed as k-major tiles so it is directly the moving
    operand of the qkv projection (out = W_tile^T . x_tile -> qkv^T).
  - cached_k passed per head as K^T [D=128, P] in slab-contiguous form: each
    [128, 2048] slab is one contiguous 1MB DMA, and every [128,128] slice is
    directly the stationary operand of the scores^T matmul.
  - cached_v passed with both heads interleaved [P, 129+129]: per-head 128
    value columns plus a constant ones column. Accumulating exp(scores^T)^T @
    [V | 1] yields attention numerator AND softmax denominator in one matmul
    (scores are O(5) here, so exp needs no max-subtraction for fp32 safety).
  - 1/sqrt(D) is folded into the q columns of w_qkv/b_qkv.
  - New-token scores use a block-diagonal mask (queries attend only their own
    batch's 16 new keys), multiplied after exp -> exact zeros off-block.
"""

import math

import numpy as np

import concourse.bass as bass
import concourse.mybir as mybir
import concourse.tile as tile
from concourse.bass_utils import run_bass_kernel_spmd
from concourse.masks import make_identity

FP = mybir.dt.float32
AFT = mybir.ActivationFunctionType

B, S, H, D = 4, 16, 16, 128
HID = H * D            # 2048
P = 8192               # cached prefix length
NQ = B * S             # 64 query tokens
NCORES = 8
HPC = H // NCORES      # heads per core = 2

KSLAB = 2048           # seq per K-slab DMA (16 chunks of 128)
VSLAB = 8              # chunks per V-slab DMA
NCHUNK = P // 128      # 64 cache chunks of 128 keys
VW = D + 1             # 129: V columns + ones column

_nc_cache = None


def _build_nc():
    nc = bass.Bass()

    xt_d = nc.declare_dram_parameter("xt", [128, 16 * NQ], FP, isOutput=False)
    wqkv_d = nc.declare_dram_parameter("wqkv", [128, 6 * 16 * 128], FP, isOutput=False)
    bqkv_d = nc.declare_dram_parameter("bqkv", [128, 6], FP, isOutput=False)
    kt_d = nc.declare_dram_parameter("kt", [2 * P // KSLAB, 128, KSLAB], FP, isOutput=False)
    vb_d = nc.declare_dram_parameter("vb", [NCHUNK // VSLAB, 128, VSLAB * 2 * VW], FP, isOutput=False)
    wp_d = nc.declare_dram_parameter("wp", [HPC, 128, HID], FP, isOutput=False)
    mask_d = nc.declare_dram_parameter("mask", [NQ, NQ], FP, isOutput=False)
    out_d = nc.declare_dram_parameter("out", [NQ, HID], FP, isOutput=True)

    with tile.TileContext(nc) as tc:
        with (
            tc.tile_pool(name="const", bufs=1) as constp,
            tc.tile_pool(name="wqkv", bufs=6) as wqp,
            tc.tile_pool(name="wproj", bufs=2) as wpp,
            tc.tile_pool(name="kslab", bufs=4) as kp,
            tc.tile_pool(name="vslab", bufs=3) as vp,
            tc.tile_pool(name="pt", bufs=8) as ptp,
            tc.tile_pool(name="small", bufs=4) as smallp,
            tc.tile_pool(name="ps_small", bufs=4, space="PSUM") as psp,
            tc.tile_pool(name="ps_acc", bufs=2, space="PSUM") as paccp,
            tc.tile_pool(name="ps_y", bufs=2, space="PSUM") as pyp,
        ):
            # ---- constants / activations ----
            xt = constp.tile([128, 16 * NQ], FP, tag="xt")
            nc.sync.dma_start(xt[:], xt_d[:])
            bq = constp.tile([128, 6], FP, tag="bq")
            nc.sync.dma_start(bq[:], bqkv_d[:])
            msk = constp.tile([NQ, NQ], FP, tag="msk")
            nc.sync.dma_start(msk[:], mask_d[:])
            ident = constp.tile([128, 128], FP, tag="ident")
            make_identity(nc, ident[:])

            wq_tiles = []
            for m in range(6):
                t_ = wqp.tile([128, 16 * 128], FP, tag="wqkv")
                nc.sync.dma_start(t_[:], wqkv_d[:, m * 2048:(m + 1) * 2048])
                wq_tiles.append(t_)

            # ---- qkv projection: qkvT[m] = (x @ Wm + bm)^T  [128, 64] ----
            # m = 0,1: q^T per head (scale pre-folded); 2,3: k^T; 4,5: v^T
            qkvT = []
            for m in range(6):
                ps = psp.tile([128, NQ], FP, tag="ps_small")
                for t in range(16):
                    nc.tensor.matmul(
                        ps[:],
                        lhsT=wq_tiles[m][:, t * 128:(t + 1) * 128],
                        rhs=xt[:, t * NQ:(t + 1) * NQ],
                        start=(t == 0),
                        stop=(t == 15),
                    )
                sb = constp.tile([128, NQ], FP, tag=f"qkvT{m}")
                nc.scalar.activation(sb[:], ps[:], AFT.Identity, bias=bq[:, m:m + 1])
                qkvT.append(sb)

            # ---- new-token attention pieces (tiny) ----
            vnew = []
            pnew = []
            for h in range(HPC):
                vt_ps = psp.tile([NQ, 128], FP, tag="ps_small")
                nc.tensor.transpose(vt_ps[:], qkvT[4 + h][:], ident[:])
                vn = constp.tile([NQ, VW], FP, tag=f"vnew{h}")
                nc.scalar.activation(vn[:, 0:128], vt_ps[:], AFT.Copy)
                nc.vector.memset(vn[:, 128:129], 1.0)
                vnew.append(vn)

                sn_ps = psp.tile([NQ, NQ], FP, tag="ps_small")
                nc.tensor.matmul(sn_ps[:], lhsT=qkvT[2 + h][:], rhs=qkvT[h][:],
                                 start=True, stop=True)
                pn = constp.tile([NQ, NQ], FP, tag=f"pn{h}")
                nc.scalar.activation(pn[:], sn_ps[:], AFT.Exp)
                pnm = constp.tile([NQ, NQ], FP, tag=f"pnm{h}")
                nc.vector.tensor_mul(pnm[:], pn[:], msk[:])
                pnew.append(pnm)

            # ---- w_proj loads (needed only at the tail) ----
            wp_tiles = []
            for h in range(HPC):
                t_ = wpp.tile([128, HID], FP, tag="wp")
                nc.sync.dma_start(t_[:], wp_d[h])
                wp_tiles.append(t_)

            # ---- main cache sweep, both heads interleaved ----
            accs = [paccp.tile([NQ, VW], FP, tag="acc") for _ in range(HPC)]
            k_sb = [None, None]
            v_sb = None
            for c in range(NCHUNK):
                if c % 16 == 0:
                    for h in range(HPC):
                        k_sb[h] = kp.tile([128, KSLAB], FP, tag="k")
                        nc.sync.dma_start(k_sb[h][:], kt_d[h * 4 + c // 16])
                if c % VSLAB == 0:
                    v_sb = vp.tile([128, VSLAB * 2 * VW], FP, tag="v")
                    nc.sync.dma_start(v_sb[:], vb_d[c // VSLAB])
                koff = (c % 16) * 128
                voff = (c % VSLAB) * 2 * VW
                for h in range(HPC):
                    s_ps = psp.tile([128, NQ], FP, tag="ps_small")
                    nc.tensor.matmul(s_ps[:], lhsT=k_sb[h][:, koff:koff + 128],
                                     rhs=qkvT[h][:], start=True, stop=True)
                    p_sb = ptp.tile([128, NQ], FP, tag="pt")
                    nc.scalar.activation(p_sb[:], s_ps[:], AFT.Exp)
                    nc.tensor.matmul(accs[h][:], lhsT=p_sb[:],
                                     rhs=v_sb[:, voff + h * VW: voff + (h + 1) * VW],
                                     start=(c == 0), stop=False)
            for h in range(HPC):
                nc.tensor.matmul(accs[h][:], lhsT=pnew[h][:], rhs=vnew[h][:],
                                 start=False, stop=True)

            # ---- normalize + transpose per head ----
            ut_tiles = []
            for h in range(HPC):
                rec = smallp.tile([NQ, 1], FP, tag="rec")
                nc.vector.reciprocal(rec[:], accs[h][:, 128:129])
                u_sb = smallp.tile([NQ, 128], FP, tag="u")
                nc.scalar.activation(u_sb[:], accs[h][:, 0:128], AFT.Copy, scale=rec[:])
                ut_ps = psp.tile([128, NQ], FP, tag="ps_small")
                nc.tensor.transpose(ut_ps[:], u_sb[:], ident[:])
                ut_sb = smallp.tile([128, NQ], FP, tag="ut")
                nc.vector.tensor_copy(ut_sb[:], ut_ps[:])
                ut_tiles.append(ut_sb)

            # ---- row-parallel output projection partial ----
            for n in range(4):
                y_ps = pyp.tile([NQ, 512], FP, tag="y")
                for h in range(HPC):
                    nc.tensor.matmul(y_ps[:], lhsT=ut_tiles[h][:],
                                     rhs=wp_tiles[h][:, n * 512:(n + 1) * 512],
                                     start=(h == 0), stop=(h == HPC - 1))
                y_sb = smallp.tile([NQ, 512], FP, tag="y_sb")
                nc.scalar.activation(y_sb[:], y_ps[:], AFT.Copy)
                nc.sync.dma_start(out_d[:, n * 512:(n + 1) * 512], y_sb[:])

    return nc


def _prep_shards(x, cached_k, cached_v, w_qkv, b_qkv, w_proj):
    scale = np.float32(1.0 / math.sqrt(D))
    x2d = np.asarray(x, np.float32).reshape(NQ, HID)
    # [feature, token] k-major tiles: host[p, t*64+q] = x2d[q? no: token q, feat t*128+p]
    xt_host = np.ascontiguousarray(
        x2d.T.reshape(16, 128, NQ).transpose(1, 0, 2).reshape(128, 16 * NQ)
    )
    mask = np.kron(np.eye(B, dtype=np.float32), np.ones((S, S), np.float32))
    mask = np.ascontiguousarray(mask)

    ck = np.asarray(cached_k, np.float32)
    cv = np.asarray(cached_v, np.float32)
    wq = np.asarray(w_qkv, np.float32)
    bq = np.asarray(b_qkv, np.float32)
    wp = np.asarray(w_proj, np.float32)

    in_maps = []
    for core in range(NCORES):
        h0 = HPC * core
        cols = slice(h0 * D, (h0 + HPC) * D)
        w_shard = np.concatenate(
            [wq[:, 0:HID][:, cols] * scale, wq[:, HID:2 * HID][:, cols],
             wq[:, 2 * HID:3 * HID][:, cols]], axis=1)          # [2048, 768]
        wqkv_host = np.ascontiguousarray(
            w_shard.reshape(16, 128, 6, 128).transpose(1, 2, 0, 3).reshape(128, 6 * 2048)
        )
        b_shard = np.concatenate(
            [bq[0:HID][cols] * scale, bq[HID:2 * HID][cols], bq[2 * HID:3 * HID][cols]])
        bqkv_host = np.ascontiguousarray(b_shard.reshape(6, 128).T)

        kt_slabs = []
        for h in (h0, h0 + 1):
            kt_h = ck[:, h, :].T                                 # [128, 8192]
            kt_slabs.append(kt_h.reshape(128, P // KSLAB, KSLAB).transpose(1, 0, 2))
        kt_host = np.ascontiguousarray(np.concatenate(kt_slabs, axis=0))

        vb = np.empty((P, 2 * VW), np.float32)
        vb[:, 0:D] = cv[:, h0, :]
        vb[:, D] = 1.0
        vb[:, VW:VW + D] = cv[:, h0 + 1, :]
        vb[:, VW + D] = 1.0
        vb_host = np.ascontiguousarray(
            vb.reshape(NCHUNK // VSLAB, VSLAB, 128, 2 * VW)
              .transpose(0, 2, 1, 3).reshape(NCHUNK // VSLAB, 128, VSLAB * 2 * VW)
        )

        wp_host = np.ascontiguousarray(
            np.stack([wp[(h0 + h) * D:(h0 + h + 1) * D, :] for h in range(HPC)])
        )

        in_maps.append({
            "xt": xt_host, "wqkv": wqkv_host, "bqkv": bqkv_host,
            "kt": kt_host, "vb": vb_host, "wp": wp_host, "mask": mask,
        })
    return in_maps


def kernel(**inputs):
    global _nc_cache
    x = np.asarray(inputs["x"], np.float32)
    b_proj = np.asarray(inputs["b_proj"], np.float32)
    in_maps = _prep_shards(
        x, inputs["cached_k"], inputs["cached_v"],
        inputs["w_qkv"], inputs["b_qkv"], inputs["w_proj"],
    )
    if _nc_cache is None:
        _nc_cache = _build_nc()
    res = run_bass_kernel_spmd(_nc_cache, in_maps, core_ids=list(range(NCORES)))
    y = np.zeros((NQ, HID), np.float64)
    for r in res.results:
        y += r["out"].astype(np.float64)
    y += b_proj.astype(np.float64)
    return y.astype(np.float32).reshape(B, S, HID)


# revision 9
# speedup vs baseline: 1.0883x; 1.0883x over previous
"""Trainium2 Bass kernel for nn_AttentionLayer (sparse_attention, 8-core head-parallel).

Reference computation (B=4, S=16, H=16, D=128, HID=2048, P=8192):
    qkv = x @ w_qkv + b_qkv ; split into q,k,v
    k_full = concat(cached_k broadcast over batch, new k)   # [B,H,P+S,D]
    out = softmax(q @ k_full^T / sqrt(D)) @ v_full
    y = out @ w_proj + b_proj

Sharding: tensor-parallel over heads. Each of the 8 cores owns 2 heads:
column-sharded w_qkv/b_qkv (its heads' q,k,v columns), the head slice of the
KV cache, and the row slice of w_proj. Each core emits a partial y
[64, 2048]; the unshard step sums the 8 partials and adds b_proj (row-parallel
linear with host-side reduction).

Device-side layout choices (prepared on host during the shard step):
  - TensorEngine-facing tensors are shipped/computed in bf16 (KV cache,
    weights, x, exp(scores)); all matmul accumulation, softmax input, the
    denominators and the output stay f32. Emulated end-to-end rel err vs the
    f32 reference is ~3e-3 (tolerance 2e-2).
  - x is passed pre-transposed k-major so it is directly the moving operand
    of the qkv projection (out = W_tile^T . x_tile -> qkv^T).
  - cached_k passed per head as K^T [D=128, P] in slab-contiguous form: each
    [128, 4096] bf16 slab is one contiguous 1MB DMA whose [128,128] slices
    are directly the stationary operand of the scores^T matmul.
  - cached_v passed with both heads interleaved [P, 129+129]: per-head 128
    value columns plus a constant ones column. Accumulating exp(scores^T)^T @
    [V | 1] yields the attention numerator AND softmax denominator in one
    matmul (scores are O(5) here, so exp needs no max-subtraction in f32).
  - 1/sqrt(D) is folded into the q columns of w_qkv/b_qkv.
  - New-token scores use a block-diagonal mask (queries attend only their own
    batch's 16 new keys), multiplied after exp -> exact zeros off-block.
  - scores^T for 4 chunks x 2 heads are packed into one [128, 512] PSUM bank
    so a single ACT instruction computes exp for all 8 score tiles.
"""

import math

import numpy as np
import ml_dtypes

import concourse.bass as bass
import concourse.mybir as mybir
import concourse.tile as tile
from concourse import bacc
from concourse.bass_utils import run_bass_kernel_spmd
from concourse.masks import make_identity

FP = mybir.dt.float32
BF = mybir.dt.bfloat16
NPBF = ml_dtypes.bfloat16
AFT = mybir.ActivationFunctionType

B, S, H, D = 4, 16, 16, 128
HID = H * D            # 2048
P = 8192               # cached prefix length
NQ = B * S             # 64 query tokens
NCORES = 8
HPC = H // NCORES      # heads per core = 2

NCHUNK = P // 128      # 64 cache chunks of 128 keys
GRP = 4                # chunks whose scores share one PSUM bank / one exp
NGRP = NCHUNK // GRP   # 16
KSLAB = 4096           # seq per K-slab DMA (32 chunks, 1MB bf16)
VSLAB = 16             # chunks per V-slab DMA (1.03MB bf16)
VW = D + 1             # 129: V columns + ones column

_nc_cache = None


def _build_nc(reps=1, loop=None):
    nc = bacc.Bacc("TRN2", target_bir_lowering=False, debug=False,
                   num_devices=NCORES)

    xt_d = nc.declare_dram_parameter("xt", [128, 16 * NQ], BF, isOutput=False)
    wqkv_d = nc.declare_dram_parameter("wqkv", [128, 6 * 16 * 128], BF, isOutput=False)
    bqkv_d = nc.declare_dram_parameter("bqkv", [128, 6], FP, isOutput=False)
    kt_d = nc.declare_dram_parameter("kt", [HPC * P // KSLAB, 128, KSLAB], BF, isOutput=False)
    vb_d = nc.declare_dram_parameter("vb", [NCHUNK // VSLAB, 128, VSLAB * 2 * VW], BF, isOutput=False)
    wp_d = nc.declare_dram_parameter("wp", [HPC, 128, HID], BF, isOutput=False)
    mask_d = nc.declare_dram_parameter("mask", [NQ, NQ], BF, isOutput=False)
    out_d = nc.declare_dram_parameter("out", [NQ, HID], FP, isOutput=True)

    with tile.TileContext(nc) as tc:
        with (
            tc.tile_pool(name="const", bufs=1) as constp,
            tc.tile_pool(name="wqkv", bufs=3) as wqp,
            tc.tile_pool(name="wproj", bufs=2) as wpp,
            tc.tile_pool(name="kslab", bufs=4) as kp,
            tc.tile_pool(name="vslab", bufs=3) as vp,
            tc.tile_pool(name="pt", bufs=4) as ptp,
            tc.tile_pool(name="small", bufs=4) as smallp,
            tc.tile_pool(name="ps_s", bufs=3, space="PSUM") as pssp,
            tc.tile_pool(name="ps_acc", bufs=2, space="PSUM") as paccp,
            tc.tile_pool(name="ps_gp", bufs=2, space="PSUM") as pgpp,
            tc.tile_pool(name="ps_misc", bufs=1, space="PSUM") as pmiscp,
        ):
            ident = constp.tile([128, 128], BF, tag="ident")
            make_identity(nc, ident[:])

            def emit(r):
                # ---- constant loads ----
                xt = constp.tile([128, 16 * NQ], BF, tag="xt", name=f"xt{r}")
                nc.sync.dma_start(xt[:], xt_d[:])
                bq = constp.tile([128, 6], FP, tag="bq", name=f"bq{r}")
                nc.sync.dma_start(bq[:], bqkv_d[:])
                msk = constp.tile([NQ, NQ], BF, tag="msk", name=f"msk{r}")
                nc.sync.dma_start(msk[:], mask_d[:])

                wq_tiles = []
                for w2 in range(3):
                    t_ = wqp.tile([128, 4096], BF, tag="wqkv", name=f"wq{w2}{r}")
                    nc.sync.dma_start(t_[:], wqkv_d[:, w2 * 4096:(w2 + 1) * 4096])
                    wq_tiles.append(t_)

                # ---- qkv projection: qkvT[m] = (x @ Wm + bm)^T  [128, 64] bf16
                # m = 0,1: q^T per head (scale pre-folded); 2,3: k^T; 4,5: v^T
                qkvT = []
                for m in range(6):
                    ps = pgpp.tile([128, NQ], FP, tag="gp", name=f"qkvps{m}{r}")
                    for t in range(16):
                        nc.tensor.matmul(
                            ps[:],
                            lhsT=wq_tiles[m // 2][:, (m % 2) * 2048 + t * 128:(m % 2) * 2048 + (t + 1) * 128],
                            rhs=xt[:, t * NQ:(t + 1) * NQ],
                            start=(t == 0),
                            stop=(t == 15),
                        )
                    sb = constp.tile([128, NQ], BF, tag=f"qkvT{m}", name=f"qkvT{m}{r}")
                    nc.scalar.activation(sb[:], ps[:], AFT.Identity, bias=bq[:, m:m + 1])
                    qkvT.append(sb)

                # ---- new-token attention pieces (tiny) ----
                vnew = []
                pnew = []
                for h in range(HPC):
                    vt_ps = pmiscp.tile([NQ, 128], BF, tag="misc", name=f"vtps{h}{r}")
                    nc.tensor.transpose(vt_ps[:], qkvT[4 + h][:], ident[:])
                    vn = constp.tile([NQ, VW], BF, tag=f"vnew{h}", name=f"vnew{h}{r}")
                    nc.scalar.activation(vn[:, 0:128], vt_ps[:], AFT.Copy)
                    nc.vector.memset(vn[:, 128:129], 1.0)
                    vnew.append(vn)

                    sn_ps = pmiscp.tile([NQ, NQ], FP, tag="misc", name=f"snps{h}{r}")
                    nc.tensor.matmul(sn_ps[:], lhsT=qkvT[2 + h][:], rhs=qkvT[h][:],
                                     start=True, stop=True)
                    pn = constp.tile([NQ, NQ], BF, tag=f"pn{h}", name=f"pn{h}{r}")
                    nc.scalar.activation(pn[:], sn_ps[:], AFT.Exp)
                    pnm = constp.tile([NQ, NQ], BF, tag=f"pnm{h}", name=f"pnm{h}{r}")
                    nc.vector.tensor_mul(pnm[:], pn[:], msk[:])
                    pnew.append(pnm)

                # ---- w_proj loads (needed only at the tail) ----
                wp_tiles = []
                for h in range(HPC):
                    t_ = wpp.tile([128, HID], BF, tag="wp", name=f"wp{h}{r}")
                    nc.sync.dma_start(t_[:], wp_d[h])
                    wp_tiles.append(t_)

                # ---- main cache sweep, both heads interleaved ----
                accs = [paccp.tile([NQ, VW], FP, tag="acc", name=f"acc{i}{r}")
                        for i in range(HPC)]
                k_sb = [None, None]
                v_sb = None
                for g in range(NGRP):
                    c0 = g * GRP
                    if c0 % (KSLAB // 128) == 0:
                        for h in range(HPC):
                            k_sb[h] = kp.tile([128, KSLAB], BF, tag="k",
                                              name=f"k{g}_{h}{r}")
                            nc.sync.dma_start(
                                k_sb[h][:],
                                kt_d[h * (P // KSLAB) + c0 // (KSLAB // 128)])
                    if c0 % VSLAB == 0:
                        v_sb = vp.tile([128, VSLAB * 2 * VW], BF, tag="v",
                                       name=f"v{g}{r}")
                        nc.sync.dma_start(v_sb[:], vb_d[c0 // VSLAB])

                    s_ps = pssp.tile([128, GRP * HPC * NQ], FP, tag="s",
                                     name=f"s{g}{r}")
                    for c2 in range(GRP):
                        koff = ((c0 + c2) % (KSLAB // 128)) * 128
                        for h in range(HPC):
                            nc.tensor.matmul(
                                s_ps[:, (c2 * HPC + h) * NQ:(c2 * HPC + h + 1) * NQ],
                                lhsT=k_sb[h][:, koff:koff + 128],
                                rhs=qkvT[h][:], start=True, stop=True)
                    p_sb = ptp.tile([128, GRP * HPC * NQ], BF, tag="pt",
                                    name=f"p{g}{r}")
                    nc.scalar.activation(p_sb[:], s_ps[:], AFT.Exp)
                    for c2 in range(GRP):
                        voff = ((c0 + c2) % VSLAB) * 2 * VW
                        for h in range(HPC):
                            nc.tensor.matmul(
                                accs[h][:],
                                lhsT=p_sb[:, (c2 * HPC + h) * NQ:(c2 * HPC + h + 1) * NQ],
                                rhs=v_sb[:, voff + h * VW:voff + (h + 1) * VW],
                                start=(g == 0 and c2 == 0), stop=False)
                for h in range(HPC):
                    nc.tensor.matmul(accs[h][:], lhsT=pnew[h][:], rhs=vnew[h][:],
                                     start=False, stop=True)

                # ---- normalize + transpose per head ----
                ut_tiles = []
                for h in range(HPC):
                    rec = smallp.tile([NQ, 1], FP, tag="rec", name=f"rec{h}{r}")
                    nc.vector.reciprocal(rec[:], accs[h][:, 128:129])
                    u_sb = smallp.tile([NQ, 128], BF, tag="u", name=f"u{h}{r}")
                    nc.scalar.activation(u_sb[:], accs[h][:, 0:128], AFT.Copy,
                                         scale=rec[:])
                    ut_ps = pmiscp.tile([128, NQ], BF, tag="misc", name=f"utps{h}{r}")
                    nc.tensor.transpose(ut_ps[:], u_sb[:], ident[0:NQ, 0:NQ])
                    ut_sb = smallp.tile([128, NQ], BF, tag="ut", name=f"ut{h}{r}")
                    nc.vector.tensor_copy(ut_sb[:], ut_ps[:])
                    ut_tiles.append(ut_sb)

                # ---- row-parallel output projection partial ----
                for n in range(4):
                    y_ps = pgpp.tile([NQ, 512], FP, tag="gp", name=f"yps{n}{r}")
                    for h in range(HPC):
                        nc.tensor.matmul(y_ps[:], lhsT=ut_tiles[h][:],
                                         rhs=wp_tiles[h][:, n * 512:(n + 1) * 512],
                                         start=(h == 0), stop=(h == HPC - 1))
                    y_sb = smallp.tile([NQ, 512], FP, tag="y_sb", name=f"y{n}{r}")
                    nc.scalar.activation(y_sb[:], y_ps[:], AFT.Copy)
                    nc.sync.dma_start(out_d[:, n * 512:(n + 1) * 512], y_sb[:])

            if loop is None:
                for rep in range(reps):
                    emit(f"r{rep}")
            else:
                with tc.For_i(0, loop, 1,
                              hint_engines=(mybir.EngineType.PE,)):
                    emit("rl")

    nc.compile()
    return nc


def _prep_shards(x, cached_k, cached_v, w_qkv, b_qkv, w_proj):
    scale = np.float32(1.0 / math.sqrt(D))
    x2d = np.asarray(x, np.float32).reshape(NQ, HID)
    xt_host = np.ascontiguousarray(
        x2d.T.reshape(16, 128, NQ).transpose(1, 0, 2).reshape(128, 16 * NQ)
    ).astype(NPBF)
    mask = np.kron(np.eye(B, dtype=np.float32), np.ones((S, S), np.float32))
    mask = np.ascontiguousarray(mask).astype(NPBF)

    ck = np.asarray(cached_k, np.float32)
    cv = np.asarray(cached_v, np.float32)
    wq = np.asarray(w_qkv, np.float32)
    bq = np.asarray(b_qkv, np.float32)
    wp = np.asarray(w_proj, np.float32)

    in_maps = []
    for core in range(NCORES):
        h0 = HPC * core
        cols = slice(h0 * D, (h0 + HPC) * D)
        w_shard = np.concatenate(
            [wq[:, 0:HID][:, cols] * scale, wq[:, HID:2 * HID][:, cols],
             wq[:, 2 * HID:3 * HID][:, cols]], axis=1)          # [2048, 768]
        wqkv_host = np.ascontiguousarray(
            w_shard.reshape(16, 128, 6, 128).transpose(1, 2, 0, 3).reshape(128, 6 * 2048)
        ).astype(NPBF)
        b_shard = np.concatenate(
            [bq[0:HID][cols] * scale, bq[HID:2 * HID][cols], bq[2 * HID:3 * HID][cols]])
        bqkv_host = np.ascontiguousarray(b_shard.reshape(6, 128).T)

        kt_slabs = []
        for h in (h0, h0 + 1):
            kt_h = ck[:, h, :].T                                 # [128, 8192]
            kt_slabs.append(kt_h.reshape(128, P // KSLAB, KSLAB).transpose(1, 0, 2))
        kt_host = np.ascontiguousarray(np.concatenate(kt_slabs, axis=0)).astype(NPBF)

        vb = np.empty((P, 2 * VW), np.float32)
        vb[:, 0:D] = cv[:, h0, :]
        vb[:, D] = 1.0
        vb[:, VW:VW + D] = cv[:, h0 + 1, :]
        vb[:, VW + D] = 1.0
        vb_host = np.ascontiguousarray(
            vb.reshape(NCHUNK // VSLAB, VSLAB, 128, 2 * VW)
              .transpose(0, 2, 1, 3).reshape(NCHUNK // VSLAB, 128, VSLAB * 2 * VW)
        ).astype(NPBF)

        wp_host = np.ascontiguousarray(
            np.stack([wp[(h0 + h) * D:(h0 + h + 1) * D, :] for h in range(HPC)])
        ).astype(NPBF)

        in_maps.append({
            "xt": xt_host, "wqkv": wqkv_host, "bqkv": bqkv_host,
            "kt": kt_host, "vb": vb_host, "wp": wp_host, "mask": mask,
        })
    return in_maps


def kernel(**inputs):
    global _nc_cache
    x = np.asarray(inputs["x"], np.float32)
    b_proj = np.asarray(inputs["b_proj"], np.float32)
    in_maps = _prep_shards(
        x, inputs["cached_k"], inputs["cached_v"],
        inputs["w_qkv"], inputs["b_qkv"], inputs["w_proj"],
    )
    if _nc_cache is None:
        _nc_cache = _build_nc()
    res = run_bass_kernel_spmd(_nc_cache, in_maps, core_ids=list(range(NCORES)))
    y = np.zeros((NQ, HID), np.float64)
    for r in res.results:
        y += r["out"].astype(np.float64)
    y += b_proj.astype(np.float64)
    return y.astype(np.float32).reshape(B, S, HID)
